# revision 23
# baseline (speedup 1.0000x reference)
"""Trainium2 Bass kernel for nn_MultiHeadAttention_39582418600023.

Model (reference bug preserved: Q = K = V = x @ W_Q):
  qkv = x @ W_Q; q,k,v = heads(qkv)
  out = softmax(causal(q k^T) / sqrt(dh)) v  ->  ctx @ W_out + b_out

Sharding (8 cores): data-parallel over batch (4) x tensor-parallel over
head groups (2).  Core c handles batch c//2, heads (c%2)*8 .. +8
(W_Q column-parallel, W_out row-parallel); host sums the two partial
out-projections per batch and adds the bias.

Per-core device kernel:
  dtypes: everything bf16 (x, W_Q, W_out cast on host; FWL weight loads
  on the PE).  Output partials bf16 (host sums in fp32).

  1. qkvT[e,t] = W_Qc^T @ x^T (bf16 matmuls), streamed by 512-query
     column chunks so attention starts early; V obtained by PE-
     transposing qkvT 128x128 blocks into VA (ones-augmented:
     VA[t,h,64]=1 gives softmax denominators for free).
  2. Attention, head-pair outer, (query-chunk, key-block) inner: both
     heads' scores ST[k,q] land in one [128,1024] PSUM tile via two
     concurrent row-tiled matmuls (base partitions 0/64).  exp:
     ACT-engine EXP (scale=1/8) for most blocks; for late diagonal
     blocks a DVE Schraudolph bit-trick exp (one tensor_scalar
     round(A*s+B) -> int16, bitcast bf16) offloads the saturated ACT
     queue.  PT consumed immediately by PV matmuls (VA^T @ PT -> rows
     0:64 unnormalized ctxT, row 64 = softmax denominator).  Upper-tri
     0/1 mask on diagonal blocks.
  3. Denominators stage across partitions (DEN64 -> DMA -> DSTK);
     reciprocal_approx_fast + bf16 cast; normalization (K=8 bf16
     one-hot broadcast matmuls + DVE mul).  For the last query chunk
     the per-head-pair norm runs inside the attention loop so only the
     final pair's den chain sits on the critical tail.  The bf16
     out-projection runs qc-major as PE filler during the next chunk's
     attention.
"""
import os
import sys

sys.path.insert(0, "/opt/trn_rl_repo")
os.environ.setdefault("MYCRO_LOCAL_CACHE", "1")

import numpy as np

B, S, D = 4, 2048, 1024
NH, DH = 16, 64
EH = 512          # e-columns per core (8 local heads)
NHL = 8           # local heads
N_CORES = 8

# Schraudolph exp in bf16-bit space: bits = round(A*s + B), bitcast bf16
# approximates exp(s/8).  B tuned -6 from 127*128 to center the sawtooth.
SCHRA_A = 128.0 * float(np.log2(np.e)) * 0.125
SCHRA_B = 16256.0 - 6.0

_CACHE = {}


def _build():
    import concourse.mybir as mybir
    import concourse.tile as tile
    from concourse import bacc
    from concourse.masks import make_identity, make_upper_triangular

    F32 = mybir.dt.float32
    BF16 = mybir.dt.bfloat16
    I16 = mybir.dt.int16
    EXP = mybir.ActivationFunctionType.Exp
    MULT = mybir.AluOpType.mult
    ADD = mybir.AluOpType.add

    nc = bacc.Bacc(None, target_bir_lowering=False, debug=True)
    with tile.TileContext(nc) as tc:
        with tc.tile_pool(name="dram", bufs=1, space="DRAM") as dram:
            xT = dram.tile([D, S], BF16, kind="ExternalInput")     # x[b].T
            wq = dram.tile([D, EH], BF16, kind="ExternalInput")    # W_Q cols
            wo = dram.tile([EH, D], BF16, kind="ExternalInput")    # W_out rows
            sel = dram.tile([8, 4 * 128], BF16, kind="ExternalInput")
            outp = dram.tile([S, D], BF16, kind="ExternalOutput")  # partial out

            with tc.tile_pool(name="persist", bufs=1) as pp:
                # qkvT: [e-block 128, eb, t], bf16 (scores operands)
                QKVT = pp.tile([128, 4, S], BF16)
                # ones-augmented V (bf16): [t%128, tb, h, 0:64]=V, [..,64]=1
                VA = pp.tile([128, 16, NHL, DH + 1], BF16)
                # unnormalized ctxT (bf16), same layout as QKVT
                CTXT = pp.tile([128, 4, S], BF16)
                IDN = pp.tile([128, 128], BF16)
                MASK = pp.tile([128, 128], F32)   # 1 on i<=j else 0
                MASK2 = pp.tile([128, 2, 128], BF16)
                make_identity(nc, IDN[:])
                make_upper_triangular(nc, MASK[:], val=1.0, diag=True)
                nc.vector.tensor_copy(MASK2[:, 0, :], MASK[:])
                nc.vector.tensor_copy(MASK2[:, 1, :], MASK[:])
                nc.vector.memset(VA[:, :, :, DH : DH + 1], 1.0)
                # denominator staging: row 64 slots for DVE evict (same
                # partition), then DMA-scatter to head-rows of DSTK
                DEN64 = pp.tile([65, 4, 512], F32)
                DSTK = pp.tile([8, 4, 512], F32)     # [head, qc, q]
                nc.vector.memset(DSTK[:], 1.0)
                RSTKF = pp.tile([8, 4, 512], F32)    # fp32 reciprocals
                RSTK = pp.tile([8, 4, 512], BF16)    # bf16 for the matmul
                # SEL[:, jb, m]: one-hot picking recip row 2jb (m<64) or
                # 2jb+1 (m>=64) -> BC = SEL.T @ R broadcasts both heads
                SEL = pp.tile([8, 4, 128], BF16)
                nc.sync.dma_start(
                    out=SEL[:].rearrange("p a b -> p (a b)"), in_=sel[:])
                # out-proj weights (loaded after startup-critical DMAs)
                WO = pp.tile([128, 4, D], BF16)

                # --- fused pipeline, tn-major: proj(tn) -> norm+
                # outproj(qc=tn-1) -> attention(qc=tn).  PSUM budget
                # (8 banks): sc 2x2 + pv 2x1 + shared "u" ring 2x1.
                with tc.tile_pool(name="px", bufs=1) as px, \
                     tc.tile_pool(name="pt", bufs=10) as ptp, \
                     tc.tile_pool(name="po", bufs=3) as po, \
                     tc.tile_pool(name="pm", bufs=2, space="PSUM") as pm, \
                     tc.tile_pool(name="pf", bufs=2, space="PSUM") as pf, \
                     tc.tile_pool(name="ppv", bufs=2, space="PSUM") as ppv:
                    WQ = px.tile([128, 8, EH], BF16)

                    def load_wq():
                        for kc in range(8):
                            eng = (nc.scalar, nc.sync,
                                   nc.gpsimd)[(kc + 1) % 3]
                            eng.dma_start(
                                out=WQ[:, kc, :],
                                in_=wq[kc * 128 : (kc + 1) * 128, :])

                    def warmup():
                        # ~4.5us of dummy matmuls: warms the PE HAM
                        # clock gate (1.2 -> 2.4 GHz) while the x/W_Q
                        # DMAs stream in
                        wt = pf.tile([128, 128], F32, tag="f")
                        for i in range(36):
                            nc.tensor.matmul(
                                wt[:], IDN[:], IDN[:],
                                start=True, stop=True)

                    def proj_eb(tn, XT, eb):
                        ps = pf.tile([128, 512], F32, tag="f")
                        for kc in range(8):
                            nc.tensor.matmul(
                                ps[:],
                                WQ[:, kc, eb * 128 : (eb + 1) * 128],
                                XT[:, kc, :],
                                start=(kc == 0), stop=(kc == 7))
                        nc.vector.tensor_copy(
                            QKVT[:, eb, tn * 512 : (tn + 1) * 512],
                            ps[:])
                        # V = PE-transposed qkvT blocks for this e-block's
                        # 4 t-chunks (batched into one bank slot, one DVE
                        # eviction into the ones-augmented VA layout)
                        tp4 = pf.tile([128, 4, 128], BF16, tag="f")
                        for ti in range(4):
                            tb = 4 * tn + ti
                            nc.tensor.transpose(
                                tp4[:, ti, :],
                                QKVT[:, eb, tb * 128 : (tb + 1) * 128],
                                IDN[:])
                        nc.vector.tensor_copy(
                            VA[:, 4 * tn : 4 * tn + 4,
                               2 * eb : 2 * eb + 2, 0:DH],
                            tp4[:].rearrange("p t (h d) -> p t h d", h=2))

                    def proj(tn, XT):
                        for eb in range(4):
                            proj_eb(tn, XT, eb)

                    def attention(qc, jbs=range(4)):
                        qs = qc * 512
                        nkb = 4 * qc + 4
                        for jb in jbs:           # head pair block
                            qA = QKVT[0:64, jb, :]    # head 2jb  [64, S]
                            qB = QKVT[64:128, jb, :]  # head 2jb+1
                            CA = ppv.tile([65, 512], F32, tag="pv")
                            CB = ppv.tile([65, 512], F32, tag="pv")
                            pts = []

                            def pv_emit(kb):
                                pt, po_, n = pts[kb]
                                for hh, C in ((0, CA), (1, CB)):
                                    nc.tensor.matmul(
                                        C[:, po_ : po_ + n],
                                        VA[:, kb, 2 * jb + hh, :],
                                        pt[:, hh, :],
                                        start=(kb == 0),
                                        stop=(kb == nkb - 1),
                                        skip_group_check=True)

                            for kb in range(nkb):
                                k0 = kb * 128
                                q0 = max(k0, qs)
                                n = qs + 512 - q0
                                po_ = q0 - qs
                                sc = pm.tile([128, 1024], F32, tag="m")
                                nc.tensor.matmul(
                                    sc[:, 0:n],
                                    qA[:, k0 : k0 + 128],
                                    qA[:, q0 : q0 + n],
                                    start=True, stop=True)
                                nc.tensor.matmul(
                                    sc[:, 512 : 512 + n],
                                    qB[:, k0 : k0 + 128],
                                    qB[:, q0 : q0 + n],
                                    start=True, stop=True)
                                pt = ptp.tile([128, 2, n], BF16, tag="pt")
                                scv = sc[:].rearrange(
                                    "p (two n) -> p two n", two=2)[
                                    :, :, 0:n]
                                # offload some exp blocks to DVE via a
                                # Schraudolph bit-trick so ACT and DVE
                                # run concurrently: every 5th full block
                                # plus half the diagonal blocks
                                if kb < 4 * qc:
                                    use_dve = False
                                else:
                                    use_dve = qc >= 1 and \
                                        (kb - 4 * qc) in (0, 2)
                                if use_dve:
                                    nc.vector.tensor_scalar(
                                        out=pt[:].bitcast(I16),
                                        in0=scv,
                                        scalar1=SCHRA_A, scalar2=SCHRA_B,
                                        op0=MULT, op1=ADD)
                                else:
                                    nc.scalar.activation(
                                        pt[:], scv, EXP, scale=0.125)
                                if k0 >= qs:   # diagonal 128x128 block
                                    nc.vector.tensor_mul(
                                        pt[:, :, 0:128], pt[:, :, 0:128],
                                        MASK2[:])
                                pts.append((pt, po_, n))
                                if kb >= 1:
                                    pv_emit(kb - 1)
                            pv_emit(nkb - 1)
                            for hh, C in ((0, CA), (1, CB)):
                                h = 2 * jb + hh
                                slot = hh * 2 + (jb & 1)
                                nc.vector.tensor_copy(
                                    DEN64[64:65, slot, :], C[64:65, :])
                                nc.sync.dma_start(
                                    out=DSTK[h : h + 1, qc, :],
                                    in_=DEN64[64:65, slot, :])
                            for hh, C in ((0, CA), (1, CB)):
                                nc.vector.tensor_copy(
                                    CTXT[hh * 64 : hh * 64 + 64, jb,
                                         qs : qs + 512],
                                    C[0:64, :])
                            if qc == 3:
                                # norm this pair now: only jb3's chain
                                # remains on the critical tail (junk
                                # rows recip garbage-safe via memset 1)
                                with nc.allow_low_precision(
                                        reason="approx recip ok"):
                                    nc.vector.reciprocal_approx_fast(
                                        out=RSTKF[:, qc, :],
                                        in_=DSTK[:, qc, :])
                                    nc.vector.tensor_copy(
                                        RSTK[:, qc, :], RSTKF[:, qc, :])
                                norm_jb(qc, jb)

                    def norm_jb(qc, jb):
                        BC = pf.tile([128, 512], F32, tag="f")
                        nc.tensor.matmul(
                            BC[:],
                            SEL[:, jb, :],
                            RSTK[:, qc, :],
                            start=True, stop=True)
                        dst = CTXT[:, jb, qc * 512 : qc * 512 + 512]
                        nc.vector.tensor_mul(dst, dst, BC[:])

                    def norm_outproj(qc):
                        if qc < 3:
                            with nc.allow_low_precision(
                                    reason="approx recip, bf16 norm ok"):
                                nc.vector.reciprocal_approx_fast(
                                    out=RSTKF[:, qc, :],
                                    in_=DSTK[:, qc, :])
                                nc.vector.tensor_copy(
                                    RSTK[:, qc, :], RSTKF[:, qc, :])
                            for jb in range(4):
                                norm_jb(qc, jb)
                        for tb in range(4 * qc, 4 * qc + 4):
                            for nn in range(2):
                                ps = pf.tile([128, 512], F32, tag="f")
                                for eb in range(4):
                                    nc.tensor.matmul(
                                        ps[:],
                                        CTXT[:, eb,
                                             tb * 128 : (tb + 1) * 128],
                                        WO[:, eb,
                                           nn * 512 : (nn + 1) * 512],
                                        start=(eb == 0), stop=(eb == 3))
                                ob = po.tile([128, 512], BF16, tag="ob")
                                nc.vector.tensor_copy(ob[:], ps[:])
                                # alternate queues so the final chunks
                                # drain in parallel (3-way for the last
                                # chunk: ACT is idle at the tail)
                                if qc == 3:
                                    oeng = (nc.sync, nc.gpsimd,
                                            nc.scalar)[(2 * tb + nn) % 3]
                                else:
                                    oeng = nc.sync if (tb + nn) % 2 == 0 \
                                        else nc.gpsimd
                                oeng.dma_start(
                                    out=outp[tb * 128 : (tb + 1) * 128,
                                             nn * 512 : (nn + 1) * 512],
                                    in_=ob[:])

                    def load_x(tn):
                        XT = px.tile([128, 8, 512], BF16, tag="xt",
                                     bufs=2, name=f"xt{tn}")
                        for kc in range(8):
                            if tn == 0:
                                # three queues: minimize time-to-first-MM
                                eng = (nc.gpsimd, nc.scalar,
                                       nc.sync)[kc % 3]
                            else:
                                eng = nc.sync if kc % 2 == 0 \
                                    else nc.gpsimd
                            eng.dma_start(
                                out=XT[:, kc, :],
                                in_=xT[kc * 128 : (kc + 1) * 128,
                                       tn * 512 : (tn + 1) * 512])
                        return XT

                    # issue order = scheduler priority: attention(tn)
                    # first (feeds ACT), then proj(tn+1) and
                    # norm+outproj(tn-1) as PE filler for stalls.
                    XTn = load_x(0)
                    load_wq()
                    XT1 = load_x(1)   # prefetch: slot B is free
                    warmup()
                    # startup: interleave proj(0) with attention(0) per
                    # head-pair (jb needs only e-block jb) so the exp
                    # engines start ~15us earlier
                    for eb in range(4):
                        proj_eb(0, XTn, eb)
                        attention(0, jbs=[eb])
                    for tn in range(4):
                        if tn >= 1:
                            attention(tn)
                        if tn == 0:
                            XTn = XT1
                            proj(1, XTn)
                            for eb in range(4):
                                nc.gpsimd.dma_start(
                                    out=WO[:, eb, :],
                                    in_=wo[eb * 128 : (eb + 1) * 128, :])
                        elif tn < 3:
                            XTn = load_x(tn + 1)
                            proj(tn + 1, XTn)
                        if tn >= 1:
                            norm_outproj(tn - 1)
                    norm_outproj(3)
    nc.compile()
    return nc, {"xT": xT.name, "wq": wq.name, "wo": wo.name,
                "sel": sel.name, "outp": outp.name}


def _get():
    if "nc" not in _CACHE:
        _CACHE["nc"], _CACHE["names"] = _build()
    return _CACHE["nc"], _CACHE["names"]


def _run(x, W_Q, W_out, trace=False):
    import ml_dtypes
    from concourse.bass_utils import run_bass_kernel_spmd

    BF = ml_dtypes.bfloat16
    nc, nm = _get()
    sel = np.zeros((8, 4, 128), np.float32)
    for jb in range(4):
        sel[2 * jb, jb, 0:64] = 1.0
        sel[2 * jb + 1, jb, 64:128] = 1.0
    sel = np.ascontiguousarray(sel.reshape(8, 512)).astype(BF)
    in_maps = []
    for c in range(N_CORES):
        b, hg = c // 2, c % 2
        in_maps.append({
            nm["xT"]: np.ascontiguousarray(x[b].T.astype(BF)),
            nm["wq"]: np.ascontiguousarray(
                W_Q[:, hg * EH : (hg + 1) * EH].astype(BF)),
            nm["wo"]: np.ascontiguousarray(
                W_out[hg * EH : (hg + 1) * EH, :].astype(BF)),
            nm["sel"]: sel,
        })
    return run_bass_kernel_spmd(
        nc, in_maps, list(range(N_CORES)), trace=trace), nm


def kernel(x, W_Q, W_out, b_out):
    res, nm = _run(np.asarray(x), np.asarray(W_Q), np.asarray(W_out))
    bo = np.asarray(b_out, dtype=np.float32)
    out = np.empty((B, S, D), np.float32)
    for b in range(B):
        out[b] = (res.results[2 * b][nm["outp"]].astype(np.float32)
                  + res.results[2 * b + 1][nm["outp"]].astype(np.float32)
                  + bo)
    return out


# revision 24
# speedup vs baseline: 1.0198x; 1.0198x over previous
"""Trainium2 Bass kernel for nn_MultiHeadAttention_39582418600023.

Model (reference bug preserved: Q = K = V = x @ W_Q):
  qkv = x @ W_Q; q,k,v = heads(qkv)
  out = softmax(causal(q k^T) / sqrt(dh)) v  ->  ctx @ W_out + b_out

Sharding (8 cores): data-parallel over batch (4) x tensor-parallel over
head groups (2).  Core c handles batch c//2, heads (c%2)*8 .. +8
(W_Q column-parallel, W_out row-parallel); host sums the two partial
out-projections per batch and adds the bias.

Per-core device kernel:
  dtypes: everything bf16 (x, W_Q, W_out cast on host; FWL weight loads
  on the PE).  Output partials bf16 (host sums in fp32).

  1. qkvT[e,t] = W_Qc^T @ x^T (bf16 matmuls), streamed by 512-query
     column chunks so attention starts early; V obtained by PE-
     transposing qkvT 128x128 blocks into VA (ones-augmented:
     VA[t,h,64]=1 gives softmax denominators for free).
  2. Attention, head-pair outer, (query-chunk, key-block) inner: both
     heads' scores ST[k,q] land in one [128,1024] PSUM tile via two
     concurrent row-tiled matmuls (base partitions 0/64).  exp:
     ACT-engine EXP (scale=1/8) for most blocks; for late diagonal
     blocks a DVE Schraudolph bit-trick exp (one tensor_scalar
     round(A*s+B) -> int16, bitcast bf16) offloads the saturated ACT
     queue.  PT consumed immediately by PV matmuls (VA^T @ PT -> rows
     0:64 unnormalized ctxT, row 64 = softmax denominator).  Upper-tri
     0/1 mask on diagonal blocks.
  3. Denominators stage across partitions (DEN64 -> DMA -> DSTK);
     reciprocal_approx_fast + bf16 cast; normalization (K=8 bf16
     one-hot broadcast matmuls + DVE mul).  For the last query chunk
     the per-head-pair norm runs inside the attention loop so only the
     final pair's den chain sits on the critical tail.  The bf16
     out-projection runs qc-major as PE filler during the next chunk's
     attention.
"""
import os
import sys

sys.path.insert(0, "/opt/trn_rl_repo")
os.environ.setdefault("MYCRO_LOCAL_CACHE", "1")

import numpy as np

B, S, D = 4, 2048, 1024
NH, DH = 16, 64
EH = 512          # e-columns per core (8 local heads)
NHL = 8           # local heads
N_CORES = 8

# Schraudolph exp in bf16-bit space: bits = round(A*s + B), bitcast bf16
# approximates exp(s/8).  B tuned -6 from 127*128 to center the sawtooth.
SCHRA_A = 128.0 * float(np.log2(np.e)) * 0.125
SCHRA_B = 16256.0 - 6.0

_CACHE = {}


def _build():
    import concourse.mybir as mybir
    import concourse.tile as tile
    from concourse import bacc
    from concourse.masks import make_identity, make_upper_triangular

    F32 = mybir.dt.float32
    BF16 = mybir.dt.bfloat16
    I16 = mybir.dt.int16
    EXP = mybir.ActivationFunctionType.Exp
    MULT = mybir.AluOpType.mult
    ADD = mybir.AluOpType.add

    nc = bacc.Bacc(None, target_bir_lowering=False, debug=True)
    with tile.TileContext(nc) as tc:
        with tc.tile_pool(name="dram", bufs=1, space="DRAM") as dram:
            xT = dram.tile([D, S], BF16, kind="ExternalInput")     # x[b].T
            wq = dram.tile([D, EH], BF16, kind="ExternalInput")    # W_Q cols
            wo = dram.tile([EH, D], BF16, kind="ExternalInput")    # W_out rows
            sel = dram.tile([8, 4 * 128], BF16, kind="ExternalInput")
            outp = dram.tile([S, D], BF16, kind="ExternalOutput")  # partial out

            with tc.tile_pool(name="persist", bufs=1) as pp:
                # qkvT: [e-block 128, eb, t], bf16 (scores operands)
                QKVT = pp.tile([128, 4, S], BF16)
                # ones-augmented V (bf16): [t%128, tb, h, 0:64]=V, [..,64]=1
                VA = pp.tile([128, 16, NHL, DH + 1], BF16)
                # unnormalized ctxT (bf16), same layout as QKVT
                CTXT = pp.tile([128, 4, S], BF16)
                IDN = pp.tile([128, 128], BF16)
                MASK = pp.tile([128, 128], F32)   # 1 on i<=j else 0
                MASK2 = pp.tile([128, 2, 128], BF16)
                make_identity(nc, IDN[:])
                make_upper_triangular(nc, MASK[:], val=1.0, diag=True)
                nc.vector.tensor_copy(MASK2[:, 0, :], MASK[:])
                nc.vector.tensor_copy(MASK2[:, 1, :], MASK[:])
                nc.vector.memset(VA[:, :, :, DH : DH + 1], 1.0)
                # denominator staging: row 64 slots for DVE evict (same
                # partition), then DMA-scatter to head-rows of DSTK
                DEN64 = pp.tile([65, 4, 512], F32)
                DSTK = pp.tile([8, 4, 512], F32)     # [head, qc, q]
                nc.vector.memset(DSTK[:], 1.0)
                RSTKF = pp.tile([8, 4, 512], F32)    # fp32 reciprocals
                RSTK = pp.tile([8, 4, 512], BF16)    # bf16 for the matmul
                # SEL[:, jb, m]: one-hot picking recip row 2jb (m<64) or
                # 2jb+1 (m>=64) -> BC = SEL.T @ R broadcasts both heads
                SEL = pp.tile([8, 4, 128], BF16)
                nc.sync.dma_start(
                    out=SEL[:].rearrange("p a b -> p (a b)"), in_=sel[:])
                # out-proj weights (loaded after startup-critical DMAs)
                WO = pp.tile([128, 4, D], BF16)

                # --- fused pipeline, tn-major: proj(tn) -> norm+
                # outproj(qc=tn-1) -> attention(qc=tn).  PSUM budget
                # (8 banks): sc 2x2 + pv 2x1 + shared "u" ring 2x1.
                with tc.tile_pool(name="px", bufs=1) as px, \
                     tc.tile_pool(name="pt", bufs=10) as ptp, \
                     tc.tile_pool(name="po", bufs=3) as po, \
                     tc.tile_pool(name="pm", bufs=2, space="PSUM") as pm, \
                     tc.tile_pool(name="pf", bufs=2, space="PSUM") as pf, \
                     tc.tile_pool(name="ppv", bufs=2, space="PSUM") as ppv:
                    WQ = px.tile([128, 8, EH], BF16)

                    def load_wq():
                        for kc in range(8):
                            eng = (nc.scalar, nc.sync,
                                   nc.gpsimd)[(kc + 1) % 3]
                            eng.dma_start(
                                out=WQ[:, kc, :],
                                in_=wq[kc * 128 : (kc + 1) * 128, :])

                    def warmup():
                        # ~4.5us of dummy matmuls: warms the PE HAM
                        # clock gate (1.2 -> 2.4 GHz) while the x/W_Q
                        # DMAs stream in
                        wt = pf.tile([128, 128], F32, tag="f")
                        for i in range(36):
                            nc.tensor.matmul(
                                wt[:], IDN[:], IDN[:],
                                start=True, stop=True)

                    def proj_eb(tn, XT, eb):
                        ps = pf.tile([128, 512], F32, tag="f")
                        for kc in range(8):
                            nc.tensor.matmul(
                                ps[:],
                                WQ[:, kc, eb * 128 : (eb + 1) * 128],
                                XT[:, kc, :],
                                start=(kc == 0), stop=(kc == 7))
                        nc.vector.tensor_copy(
                            QKVT[:, eb, tn * 512 : (tn + 1) * 512],
                            ps[:])
                        # V = PE-transposed qkvT blocks for this e-block's
                        # 4 t-chunks (batched into one bank slot, one DVE
                        # eviction into the ones-augmented VA layout)
                        tp4 = pf.tile([128, 4, 128], BF16, tag="f")
                        for ti in range(4):
                            tb = 4 * tn + ti
                            nc.tensor.transpose(
                                tp4[:, ti, :],
                                QKVT[:, eb, tb * 128 : (tb + 1) * 128],
                                IDN[:])
                        nc.vector.tensor_copy(
                            VA[:, 4 * tn : 4 * tn + 4,
                               2 * eb : 2 * eb + 2, 0:DH],
                            tp4[:].rearrange("p t (h d) -> p t h d", h=2))

                    def proj(tn, XT):
                        for eb in range(4):
                            proj_eb(tn, XT, eb)

                    def attention(qc, jbs=range(4)):
                        qs = qc * 512
                        nkb = 4 * qc + 4
                        for jb in jbs:           # head pair block
                            qA = QKVT[0:64, jb, :]    # head 2jb  [64, S]
                            qB = QKVT[64:128, jb, :]  # head 2jb+1
                            CA = ppv.tile([65, 512], F32, tag="pv")
                            CB = ppv.tile([65, 512], F32, tag="pv")
                            pts = []

                            def pv_emit(kb):
                                pt, po_, n = pts[kb]
                                for hh, C in ((0, CA), (1, CB)):
                                    nc.tensor.matmul(
                                        C[:, po_ : po_ + n],
                                        VA[:, kb, 2 * jb + hh, :],
                                        pt[:, hh, :],
                                        start=(kb == 0),
                                        stop=(kb == nkb - 1),
                                        skip_group_check=True)

                            for kb in range(nkb):
                                k0 = kb * 128
                                q0 = max(k0, qs)
                                n = qs + 512 - q0
                                po_ = q0 - qs
                                sc = pm.tile([128, 1024], F32, tag="m")
                                nc.tensor.matmul(
                                    sc[:, 0:n],
                                    qA[:, k0 : k0 + 128],
                                    qA[:, q0 : q0 + n],
                                    start=True, stop=True)
                                nc.tensor.matmul(
                                    sc[:, 512 : 512 + n],
                                    qB[:, k0 : k0 + 128],
                                    qB[:, q0 : q0 + n],
                                    start=True, stop=True)
                                pt = ptp.tile([128, 2, n], BF16, tag="pt")
                                scv = sc[:].rearrange(
                                    "p (two n) -> p two n", two=2)[
                                    :, :, 0:n]
                                # offload some exp blocks to DVE via a
                                # Schraudolph bit-trick so ACT and DVE
                                # run concurrently: every 5th full block
                                # plus half the diagonal blocks
                                use_dve = qc >= 1 and kb >= 4 * qc
                                if use_dve:
                                    nc.vector.tensor_scalar(
                                        out=pt[:].bitcast(I16),
                                        in0=scv,
                                        scalar1=SCHRA_A, scalar2=SCHRA_B,
                                        op0=MULT, op1=ADD)
                                else:
                                    nc.scalar.activation(
                                        pt[:], scv, EXP, scale=0.125)
                                if k0 >= qs:   # diagonal 128x128 block
                                    nc.vector.tensor_mul(
                                        pt[:, :, 0:128], pt[:, :, 0:128],
                                        MASK2[:])
                                pts.append((pt, po_, n))
                                if kb >= 1:
                                    pv_emit(kb - 1)
                            pv_emit(nkb - 1)
                            for hh, C in ((0, CA), (1, CB)):
                                h = 2 * jb + hh
                                slot = hh * 2 + (jb & 1)
                                nc.vector.tensor_copy(
                                    DEN64[64:65, slot, :], C[64:65, :])
                                nc.sync.dma_start(
                                    out=DSTK[h : h + 1, qc, :],
                                    in_=DEN64[64:65, slot, :])
                            for hh, C in ((0, CA), (1, CB)):
                                nc.vector.tensor_copy(
                                    CTXT[hh * 64 : hh * 64 + 64, jb,
                                         qs : qs + 512],
                                    C[0:64, :])
                            if qc == 3:
                                # norm this pair now: only jb3's chain
                                # remains on the critical tail (junk
                                # rows recip garbage-safe via memset 1)
                                with nc.allow_low_precision(
                                        reason="approx recip ok"):
                                    nc.vector.reciprocal_approx_fast(
                                        out=RSTKF[:, qc, :],
                                        in_=DSTK[:, qc, :])
                                    nc.vector.tensor_copy(
                                        RSTK[:, qc, :], RSTKF[:, qc, :])
                                norm_jb(qc, jb)

                    def norm_jb(qc, jb):
                        BC = pf.tile([128, 512], F32, tag="f")
                        nc.tensor.matmul(
                            BC[:],
                            SEL[:, jb, :],
                            RSTK[:, qc, :],
                            start=True, stop=True)
                        dst = CTXT[:, jb, qc * 512 : qc * 512 + 512]
                        nc.vector.tensor_mul(dst, dst, BC[:])

                    def norm_outproj(qc):
                        if qc < 3:
                            with nc.allow_low_precision(
                                    reason="approx recip, bf16 norm ok"):
                                nc.vector.reciprocal_approx_fast(
                                    out=RSTKF[:, qc, :],
                                    in_=DSTK[:, qc, :])
                                nc.vector.tensor_copy(
                                    RSTK[:, qc, :], RSTKF[:, qc, :])
                            for jb in range(4):
                                norm_jb(qc, jb)
                        for tb in range(4 * qc, 4 * qc + 4):
                            for nn in range(2):
                                ps = pf.tile([128, 512], F32, tag="f")
                                for eb in range(4):
                                    nc.tensor.matmul(
                                        ps[:],
                                        CTXT[:, eb,
                                             tb * 128 : (tb + 1) * 128],
                                        WO[:, eb,
                                           nn * 512 : (nn + 1) * 512],
                                        start=(eb == 0), stop=(eb == 3))
                                ob = po.tile([128, 512], BF16, tag="ob")
                                nc.vector.tensor_copy(ob[:], ps[:])
                                # alternate queues so the final chunks
                                # drain in parallel (3-way for the last
                                # chunk: ACT is idle at the tail)
                                if qc == 3:
                                    oeng = (nc.sync, nc.gpsimd,
                                            nc.scalar)[(2 * tb + nn) % 3]
                                else:
                                    oeng = nc.sync if (tb + nn) % 2 == 0 \
                                        else nc.gpsimd
                                oeng.dma_start(
                                    out=outp[tb * 128 : (tb + 1) * 128,
                                             nn * 512 : (nn + 1) * 512],
                                    in_=ob[:])

                    def load_x(tn):
                        XT = px.tile([128, 8, 512], BF16, tag="xt",
                                     bufs=2, name=f"xt{tn}")
                        for kc in range(8):
                            if tn == 0:
                                # three queues: minimize time-to-first-MM
                                eng = (nc.gpsimd, nc.scalar,
                                       nc.sync)[kc % 3]
                            else:
                                eng = nc.sync if kc % 2 == 0 \
                                    else nc.gpsimd
                            eng.dma_start(
                                out=XT[:, kc, :],
                                in_=xT[kc * 128 : (kc + 1) * 128,
                                       tn * 512 : (tn + 1) * 512])
                        return XT

                    # issue order = scheduler priority: attention(tn)
                    # first (feeds ACT), then proj(tn+1) and
                    # norm+outproj(tn-1) as PE filler for stalls.
                    XTn = load_x(0)
                    load_wq()
                    XT1 = load_x(1)   # prefetch: slot B is free
                    warmup()
                    # startup: interleave proj(0) with attention(0) per
                    # head-pair (jb needs only e-block jb) so the exp
                    # engines start ~15us earlier
                    for eb in range(4):
                        proj_eb(0, XTn, eb)
                        attention(0, jbs=[eb])
                    for tn in range(4):
                        if tn >= 1:
                            attention(tn)
                        if tn == 0:
                            XTn = XT1
                            proj(1, XTn)
                            for eb in range(4):
                                nc.gpsimd.dma_start(
                                    out=WO[:, eb, :],
                                    in_=wo[eb * 128 : (eb + 1) * 128, :])
                        elif tn < 3:
                            XTn = load_x(tn + 1)
                            proj(tn + 1, XTn)
                        if tn >= 1:
                            norm_outproj(tn - 1)
                    norm_outproj(3)
    nc.compile()
    return nc, {"xT": xT.name, "wq": wq.name, "wo": wo.name,
                "sel": sel.name, "outp": outp.name}


def _get():
    if "nc" not in _CACHE:
        _CACHE["nc"], _CACHE["names"] = _build()
    return _CACHE["nc"], _CACHE["names"]


def _run(x, W_Q, W_out, trace=False):
    import ml_dtypes
    from concourse.bass_utils import run_bass_kernel_spmd

    BF = ml_dtypes.bfloat16
    nc, nm = _get()
    sel = np.zeros((8, 4, 128), np.float32)
    for jb in range(4):
        sel[2 * jb, jb, 0:64] = 1.0
        sel[2 * jb + 1, jb, 64:128] = 1.0
    sel = np.ascontiguousarray(sel.reshape(8, 512)).astype(BF)
    in_maps = []
    for c in range(N_CORES):
        b, hg = c // 2, c % 2
        in_maps.append({
            nm["xT"]: np.ascontiguousarray(x[b].T.astype(BF)),
            nm["wq"]: np.ascontiguousarray(
                W_Q[:, hg * EH : (hg + 1) * EH].astype(BF)),
            nm["wo"]: np.ascontiguousarray(
                W_out[hg * EH : (hg + 1) * EH, :].astype(BF)),
            nm["sel"]: sel,
        })
    return run_bass_kernel_spmd(
        nc, in_maps, list(range(N_CORES)), trace=trace), nm


def kernel(x, W_Q, W_out, b_out):
    res, nm = _run(np.asarray(x), np.asarray(W_Q), np.asarray(W_out))
    bo = np.asarray(b_out, dtype=np.float32)
    out = np.empty((B, S, D), np.float32)
    for b in range(B):
        out[b] = (res.results[2 * b][nm["outp"]].astype(np.float32)
                  + res.results[2 * b + 1][nm["outp"]].astype(np.float32)
                  + bo)
    return out


# revision 25
# speedup vs baseline: 1.0609x; 1.0403x over previous
"""Trainium2 Bass kernel for nn_MultiHeadAttention_39582418600023.

Model (reference bug preserved: Q = K = V = x @ W_Q):
  qkv = x @ W_Q; q,k,v = heads(qkv)
  out = softmax(causal(q k^T) / sqrt(dh)) v  ->  ctx @ W_out + b_out

Sharding (8 cores): data-parallel over batch (4) x tensor-parallel over
head groups (2).  Core c handles batch c//2, heads (c%2)*8 .. +8
(W_Q column-parallel, W_out row-parallel); host sums the two partial
out-projections per batch and adds the bias.

Per-core device kernel:
  dtypes: everything bf16 (x, W_Q, W_out cast on host; FWL weight loads
  on the PE).  Output partials bf16 (host sums in fp32).

  1. qkvT[e,t] = W_Qc^T @ x^T (bf16 matmuls), streamed by 512-query
     column chunks so attention starts early; V obtained by PE-
     transposing qkvT 128x128 blocks into VA (ones-augmented:
     VA[t,h,64]=1 gives softmax denominators for free).
  2. Attention, head-pair outer, (query-chunk, key-block) inner: both
     heads' scores ST[k,q] land in one [128,1024] PSUM tile via two
     concurrent row-tiled matmuls (base partitions 0/64).  exp:
     ACT-engine EXP (scale=1/8) for most blocks; for late diagonal
     blocks a DVE Schraudolph bit-trick exp (one tensor_scalar
     round(A*s+B) -> int16, bitcast bf16) offloads the saturated ACT
     queue.  PT consumed immediately by PV matmuls (VA^T @ PT -> rows
     0:64 unnormalized ctxT, row 64 = softmax denominator).  Upper-tri
     0/1 mask on diagonal blocks.
  3. Denominators stage across partitions (DEN64 -> DMA -> DSTK);
     reciprocal_approx_fast + bf16 cast; normalization (K=8 bf16
     one-hot broadcast matmuls + DVE mul).  For the last query chunk
     the per-head-pair norm runs inside the attention loop so only the
     final pair's den chain sits on the critical tail.  The bf16
     out-projection runs qc-major as PE filler during the next chunk's
     attention.
"""
import os
import sys

sys.path.insert(0, "/opt/trn_rl_repo")
os.environ.setdefault("MYCRO_LOCAL_CACHE", "1")

import numpy as np

B, S, D = 4, 2048, 1024
NH, DH = 16, 64
EH = 512          # e-columns per core (8 local heads)
NHL = 8           # local heads
N_CORES = 8

# Schraudolph exp in bf16-bit space: bits = round(A*s + B), bitcast bf16
# approximates exp(s/8).  B tuned -6 from 127*128 to center the sawtooth.
SCHRA_A = 128.0 * float(np.log2(np.e)) * 0.125
SCHRA_B = 16256.0 - 6.0

_CACHE = {}


def _build():
    import concourse.mybir as mybir
    import concourse.tile as tile
    from concourse import bacc
    from concourse.masks import make_identity, make_upper_triangular

    F32 = mybir.dt.float32
    BF16 = mybir.dt.bfloat16
    I16 = mybir.dt.int16
    EXP = mybir.ActivationFunctionType.Exp
    MULT = mybir.AluOpType.mult
    ADD = mybir.AluOpType.add

    nc = bacc.Bacc(None, target_bir_lowering=False, debug=True)
    with tile.TileContext(nc) as tc:
        with tc.tile_pool(name="dram", bufs=1, space="DRAM") as dram:
            xT = dram.tile([D, S], BF16, kind="ExternalInput")     # x[b].T
            wq = dram.tile([D, EH], BF16, kind="ExternalInput")    # W_Q cols
            wo = dram.tile([EH, D], BF16, kind="ExternalInput")    # W_out rows
            sel = dram.tile([8, 4 * 128], BF16, kind="ExternalInput")
            outp = dram.tile([S, D], BF16, kind="ExternalOutput")  # partial out

            with tc.tile_pool(name="persist", bufs=1) as pp:
                # qkvT: [e-block 128, eb, t], bf16 (scores operands)
                QKVT = pp.tile([128, 4, S], BF16)
                # ones-augmented V (bf16): [t%128, tb, h, 0:64]=V, [..,64]=1
                VA = pp.tile([128, 16, NHL, DH + 1], BF16)
                # unnormalized ctxT (bf16), same layout as QKVT
                CTXT = pp.tile([128, 4, S], BF16)
                IDN = pp.tile([128, 128], BF16)
                MASK = pp.tile([128, 128], F32)   # 1 on i<=j else 0
                MASK2 = pp.tile([128, 2, 128], BF16)
                make_identity(nc, IDN[:])
                make_upper_triangular(nc, MASK[:], val=1.0, diag=True)
                nc.vector.tensor_copy(MASK2[:, 0, :], MASK[:])
                nc.vector.tensor_copy(MASK2[:, 1, :], MASK[:])
                nc.vector.memset(VA[:, :, :, DH : DH + 1], 1.0)
                # denominator staging: row 64 slots for DVE evict (same
                # partition), then DMA-scatter to head-rows of DSTK
                DEN64 = pp.tile([65, 4, 512], F32)
                DSTK = pp.tile([8, 4, 512], F32)     # [head, qc, q]
                nc.vector.memset(DSTK[:], 1.0)
                RSTKF = pp.tile([8, 4, 512], F32)    # fp32 reciprocals
                RSTK = pp.tile([8, 4, 512], BF16)    # bf16 for the matmul
                # SEL[:, jb, m]: one-hot picking recip row 2jb (m<64) or
                # 2jb+1 (m>=64) -> BC = SEL.T @ R broadcasts both heads
                SEL = pp.tile([8, 4, 128], BF16)
                nc.sync.dma_start(
                    out=SEL[:].rearrange("p a b -> p (a b)"), in_=sel[:])
                # out-proj weights (loaded after startup-critical DMAs)
                WO = pp.tile([128, 4, D], BF16)

                # --- fused pipeline, tn-major: proj(tn) -> norm+
                # outproj(qc=tn-1) -> attention(qc=tn).  PSUM budget
                # (8 banks): sc 2x2 + pv 2x1 + shared "u" ring 2x1.
                with tc.tile_pool(name="px", bufs=1) as px, \
                     tc.tile_pool(name="pt", bufs=10) as ptp, \
                     tc.tile_pool(name="po", bufs=3) as po, \
                     tc.tile_pool(name="pm", bufs=2, space="PSUM") as pm, \
                     tc.tile_pool(name="pf", bufs=2, space="PSUM") as pf, \
                     tc.tile_pool(name="ppv", bufs=2, space="PSUM") as ppv:
                    WQ = px.tile([128, 8, EH], BF16)

                    def load_wq():
                        for kc in range(8):
                            eng = (nc.scalar, nc.sync,
                                   nc.gpsimd)[(kc + 1) % 3]
                            eng.dma_start(
                                out=WQ[:, kc, :],
                                in_=wq[kc * 128 : (kc + 1) * 128, :])

                    def warmup():
                        # ~4.5us of dummy matmuls: warms the PE HAM
                        # clock gate (1.2 -> 2.4 GHz) while the x/W_Q
                        # DMAs stream in
                        wt = pf.tile([128, 128], F32, tag="f")
                        for i in range(36):
                            nc.tensor.matmul(
                                wt[:], IDN[:], IDN[:],
                                start=True, stop=True)

                    def proj_eb(tn, XT, eb):
                        ps = pf.tile([128, 512], F32, tag="f")
                        for kc in range(8):
                            nc.tensor.matmul(
                                ps[:],
                                WQ[:, kc, eb * 128 : (eb + 1) * 128],
                                XT[:, kc, :],
                                start=(kc == 0), stop=(kc == 7))
                        nc.vector.tensor_copy(
                            QKVT[:, eb, tn * 512 : (tn + 1) * 512],
                            ps[:])
                        # V = PE-transposed qkvT blocks for this e-block's
                        # 4 t-chunks (batched into one bank slot, one DVE
                        # eviction into the ones-augmented VA layout)
                        tp4 = pf.tile([128, 4, 128], BF16, tag="f")
                        for ti in range(4):
                            tb = 4 * tn + ti
                            nc.tensor.transpose(
                                tp4[:, ti, :],
                                QKVT[:, eb, tb * 128 : (tb + 1) * 128],
                                IDN[:])
                        nc.vector.tensor_copy(
                            VA[:, 4 * tn : 4 * tn + 4,
                               2 * eb : 2 * eb + 2, 0:DH],
                            tp4[:].rearrange("p t (h d) -> p t h d", h=2))

                    def proj(tn, XT):
                        for eb in range(4):
                            proj_eb(tn, XT, eb)

                    def attention(qc, jbs=range(4)):
                        qs = qc * 512
                        nkb = 4 * qc + 4
                        for jb in jbs:           # head pair block
                            qA = QKVT[0:64, jb, :]    # head 2jb  [64, S]
                            qB = QKVT[64:128, jb, :]  # head 2jb+1
                            CA = ppv.tile([65, 512], F32, tag="pv")
                            CB = ppv.tile([65, 512], F32, tag="pv")
                            pts = []

                            def pv_emit(kb):
                                pt, po_, n = pts[kb]
                                for hh, C in ((0, CA), (1, CB)):
                                    nc.tensor.matmul(
                                        C[:, po_ : po_ + n],
                                        VA[:, kb, 2 * jb + hh, :],
                                        pt[:, hh, :],
                                        start=(kb == 0),
                                        stop=(kb == nkb - 1),
                                        skip_group_check=True)

                            for kb in range(nkb):
                                k0 = kb * 128
                                q0 = max(k0, qs)
                                n = qs + 512 - q0
                                po_ = q0 - qs
                                sc = pm.tile([128, 1024], F32, tag="m")
                                nc.tensor.matmul(
                                    sc[:, 0:n],
                                    qA[:, k0 : k0 + 128],
                                    qA[:, q0 : q0 + n],
                                    start=True, stop=True)
                                nc.tensor.matmul(
                                    sc[:, 512 : 512 + n],
                                    qB[:, k0 : k0 + 128],
                                    qB[:, q0 : q0 + n],
                                    start=True, stop=True)
                                pt = ptp.tile([128, 2, n], BF16, tag="pt")
                                scv = sc[:].rearrange(
                                    "p (two n) -> p two n", two=2)[
                                    :, :, 0:n]
                                # offload some exp blocks to DVE via a
                                # Schraudolph bit-trick so ACT and DVE
                                # run concurrently: every 5th full block
                                # plus half the diagonal blocks
                                use_dve = qc >= 1 and kb >= 4 * qc
                                if use_dve:
                                    nc.vector.tensor_scalar(
                                        out=pt[:].bitcast(I16),
                                        in0=scv,
                                        scalar1=SCHRA_A, scalar2=SCHRA_B,
                                        op0=MULT, op1=ADD)
                                else:
                                    nc.scalar.activation(
                                        pt[:], scv, EXP, scale=0.125)
                                if k0 >= qs:   # diagonal 128x128 block
                                    nc.vector.tensor_mul(
                                        pt[:, :, 0:128], pt[:, :, 0:128],
                                        MASK2[:])
                                pts.append((pt, po_, n))
                                if kb >= 1:
                                    pv_emit(kb - 1)
                            pv_emit(nkb - 1)
                            for hh, C in ((0, CA), (1, CB)):
                                h = 2 * jb + hh
                                slot = hh * 2 + (jb & 1)
                                nc.vector.tensor_copy(
                                    DEN64[64:65, slot, :], C[64:65, :])
                                nc.sync.dma_start(
                                    out=DSTK[h : h + 1, qc, :],
                                    in_=DEN64[64:65, slot, :])
                            for hh, C in ((0, CA), (1, CB)):
                                nc.vector.tensor_copy(
                                    CTXT[hh * 64 : hh * 64 + 64, jb,
                                         qs : qs + 512],
                                    C[0:64, :])

                    def norm_jb(qc, jb):
                        BC = pf.tile([128, 512], F32, tag="f")
                        nc.tensor.matmul(
                            BC[:],
                            SEL[:, jb, :],
                            RSTK[:, qc, :],
                            start=True, stop=True)
                        dst = CTXT[:, jb, qc * 512 : qc * 512 + 512]
                        nc.vector.tensor_mul(dst, dst, BC[:])

                    def norm_outproj(qc):
                        if True:
                            with nc.allow_low_precision(
                                    reason="approx recip, bf16 norm ok"):
                                nc.vector.reciprocal_approx_fast(
                                    out=RSTKF[:, qc, :],
                                    in_=DSTK[:, qc, :])
                                nc.vector.tensor_copy(
                                    RSTK[:, qc, :], RSTKF[:, qc, :])
                            for jb in range(4):
                                norm_jb(qc, jb)
                        for tb in range(4 * qc, 4 * qc + 4):
                            for nn in range(2):
                                ps = pf.tile([128, 512], F32, tag="f")
                                for eb in range(4):
                                    nc.tensor.matmul(
                                        ps[:],
                                        CTXT[:, eb,
                                             tb * 128 : (tb + 1) * 128],
                                        WO[:, eb,
                                           nn * 512 : (nn + 1) * 512],
                                        start=(eb == 0), stop=(eb == 3))
                                ob = po.tile([128, 512], BF16, tag="ob")
                                nc.vector.tensor_copy(ob[:], ps[:])
                                # alternate queues so the final chunks
                                # drain in parallel (3-way for the last
                                # chunk: ACT is idle at the tail)
                                if qc == 3:
                                    oeng = (nc.sync, nc.gpsimd,
                                            nc.scalar)[(2 * tb + nn) % 3]
                                else:
                                    oeng = nc.sync if (tb + nn) % 2 == 0 \
                                        else nc.gpsimd
                                oeng.dma_start(
                                    out=outp[tb * 128 : (tb + 1) * 128,
                                             nn * 512 : (nn + 1) * 512],
                                    in_=ob[:])

                    def load_x(tn):
                        XT = px.tile([128, 8, 512], BF16, tag="xt",
                                     bufs=2, name=f"xt{tn}")
                        for kc in range(8):
                            if tn == 0:
                                # three queues: minimize time-to-first-MM
                                eng = (nc.gpsimd, nc.scalar,
                                       nc.sync)[kc % 3]
                            else:
                                eng = nc.sync if kc % 2 == 0 \
                                    else nc.gpsimd
                            eng.dma_start(
                                out=XT[:, kc, :],
                                in_=xT[kc * 128 : (kc + 1) * 128,
                                       tn * 512 : (tn + 1) * 512])
                        return XT

                    # issue order = scheduler priority: attention(tn)
                    # first (feeds ACT), then proj(tn+1) and
                    # norm+outproj(tn-1) as PE filler for stalls.
                    XTn = load_x(0)
                    load_wq()
                    XT1 = load_x(1)   # prefetch: slot B is free
                    warmup()
                    # startup: interleave proj(0) with attention(0) per
                    # head-pair (jb needs only e-block jb) so the exp
                    # engines start ~15us earlier
                    for eb in range(4):
                        proj_eb(0, XTn, eb)
                        attention(0, jbs=[eb])
                    for tn in range(4):
                        if tn >= 1:
                            attention(tn)
                        if tn == 0:
                            XTn = XT1
                            proj(1, XTn)
                            for eb in range(4):
                                nc.gpsimd.dma_start(
                                    out=WO[:, eb, :],
                                    in_=wo[eb * 128 : (eb + 1) * 128, :])
                        elif tn < 3:
                            XTn = load_x(tn + 1)
                            proj(tn + 1, XTn)
                        if tn >= 1:
                            norm_outproj(tn - 1)
                    norm_outproj(3)
    nc.compile()
    return nc, {"xT": xT.name, "wq": wq.name, "wo": wo.name,
                "sel": sel.name, "outp": outp.name}


def _get():
    if "nc" not in _CACHE:
        _CACHE["nc"], _CACHE["names"] = _build()
    return _CACHE["nc"], _CACHE["names"]


def _run(x, W_Q, W_out, trace=False):
    import ml_dtypes
    from concourse.bass_utils import run_bass_kernel_spmd

    BF = ml_dtypes.bfloat16
    nc, nm = _get()
    sel = np.zeros((8, 4, 128), np.float32)
    for jb in range(4):
        sel[2 * jb, jb, 0:64] = 1.0
        sel[2 * jb + 1, jb, 64:128] = 1.0
    sel = np.ascontiguousarray(sel.reshape(8, 512)).astype(BF)
    in_maps = []
    for c in range(N_CORES):
        b, hg = c // 2, c % 2
        in_maps.append({
            nm["xT"]: np.ascontiguousarray(x[b].T.astype(BF)),
            nm["wq"]: np.ascontiguousarray(
                W_Q[:, hg * EH : (hg + 1) * EH].astype(BF)),
            nm["wo"]: np.ascontiguousarray(
                W_out[hg * EH : (hg + 1) * EH, :].astype(BF)),
            nm["sel"]: sel,
        })
    return run_bass_kernel_spmd(
        nc, in_maps, list(range(N_CORES)), trace=trace), nm


def kernel(x, W_Q, W_out, b_out):
    res, nm = _run(np.asarray(x), np.asarray(W_Q), np.asarray(W_out))
    bo = np.asarray(b_out, dtype=np.float32)
    out = np.empty((B, S, D), np.float32)
    for b in range(B):
        out[b] = (res.results[2 * b][nm["outp"]].astype(np.float32)
                  + res.results[2 * b + 1][nm["outp"]].astype(np.float32)
                  + bo)
    return out


# revision 26
# speedup vs baseline: 1.0795x; 1.0176x over previous
"""Trainium2 Bass kernel for nn_MultiHeadAttention_39582418600023.

Model (reference bug preserved: Q = K = V = x @ W_Q):
  qkv = x @ W_Q; q,k,v = heads(qkv)
  out = softmax(causal(q k^T) / sqrt(dh)) v  ->  ctx @ W_out + b_out

Sharding (8 cores): data-parallel over batch (4) x tensor-parallel over
head groups (2).  Core c handles batch c//2, heads (c%2)*8 .. +8
(W_Q column-parallel, W_out row-parallel); host sums the two partial
out-projections per batch and adds the bias.

Per-core device kernel:
  dtypes: everything bf16 (x, W_Q, W_out cast on host; FWL weight loads
  on the PE).  Output partials bf16 (host sums in fp32).

  1. qkvT[e,t] = W_Qc^T @ x^T (bf16 matmuls), streamed by 512-query
     column chunks so attention starts early; V obtained by PE-
     transposing qkvT 128x128 blocks into VA (ones-augmented:
     VA[t,h,64]=1 gives softmax denominators for free).
  2. Attention, head-pair outer, (query-chunk, key-block) inner: both
     heads' scores ST[k,q] land in one [128,1024] PSUM tile via two
     concurrent row-tiled matmuls (base partitions 0/64).  exp:
     ACT-engine EXP (scale=1/8) for most blocks; for late diagonal
     blocks a DVE Schraudolph bit-trick exp (one tensor_scalar
     round(A*s+B) -> int16, bitcast bf16) offloads the saturated ACT
     queue.  PT consumed immediately by PV matmuls (VA^T @ PT -> rows
     0:64 unnormalized ctxT, row 64 = softmax denominator).  Upper-tri
     0/1 mask on diagonal blocks.
  3. Denominators stage across partitions (DEN64 -> DMA -> DSTK);
     reciprocal_approx_fast + bf16 cast; normalization (K=8 bf16
     one-hot broadcast matmuls + DVE mul).  For the last query chunk
     the per-head-pair norm runs inside the attention loop so only the
     final pair's den chain sits on the critical tail.  The bf16
     out-projection runs qc-major as PE filler during the next chunk's
     attention.
"""
import os
import sys

sys.path.insert(0, "/opt/trn_rl_repo")
os.environ.setdefault("MYCRO_LOCAL_CACHE", "1")

import numpy as np

B, S, D = 4, 2048, 1024
NH, DH = 16, 64
EH = 512          # e-columns per core (8 local heads)
NHL = 8           # local heads
N_CORES = 8

# Schraudolph exp in bf16-bit space: bits = round(A*s + B), bitcast bf16
# approximates exp(s/8).  B tuned -6 from 127*128 to center the sawtooth.
SCHRA_A = 128.0 * float(np.log2(np.e)) * 0.125
SCHRA_B = 16256.0 - 6.0

_CACHE = {}


def _build():
    import concourse.mybir as mybir
    import concourse.tile as tile
    from concourse import bacc
    from concourse.masks import make_identity, make_upper_triangular

    F32 = mybir.dt.float32
    BF16 = mybir.dt.bfloat16
    I16 = mybir.dt.int16
    EXP = mybir.ActivationFunctionType.Exp
    MULT = mybir.AluOpType.mult
    ADD = mybir.AluOpType.add

    nc = bacc.Bacc(None, target_bir_lowering=False, debug=True)
    with tile.TileContext(nc) as tc:
        with tc.tile_pool(name="dram", bufs=1, space="DRAM") as dram:
            xT = dram.tile([D, S], BF16, kind="ExternalInput")     # x[b].T
            wq = dram.tile([D, EH], BF16, kind="ExternalInput")    # W_Q cols
            wo = dram.tile([EH, D], BF16, kind="ExternalInput")    # W_out rows
            sel = dram.tile([8, 4 * 128], BF16, kind="ExternalInput")
            outp = dram.tile([S, D], BF16, kind="ExternalOutput")  # partial out

            with tc.tile_pool(name="persist", bufs=1) as pp:
                # qkvT: [e-block 128, eb, t], bf16 (scores operands)
                QKVT = pp.tile([128, 4, S], BF16)
                # ones-augmented V (bf16): [t%128, tb, h, 0:64]=V, [..,64]=1
                VA = pp.tile([128, 16, NHL, DH + 1], BF16)
                # unnormalized ctxT (bf16), same layout as QKVT
                CTXT = pp.tile([128, 4, S], BF16)
                IDN = pp.tile([128, 128], BF16)
                MASK = pp.tile([128, 128], F32)   # 1 on i<=j else 0
                MASK2 = pp.tile([128, 2, 128], BF16)
                make_identity(nc, IDN[:])
                make_upper_triangular(nc, MASK[:], val=1.0, diag=True)
                nc.vector.tensor_copy(MASK2[:, 0, :], MASK[:])
                nc.vector.tensor_copy(MASK2[:, 1, :], MASK[:])
                nc.vector.memset(VA[:, :, :, DH : DH + 1], 1.0)
                # denominator staging: row 64 slots for DVE evict (same
                # partition), then DMA-scatter to head-rows of DSTK
                DEN64 = pp.tile([65, 4, 512], F32)
                DSTK = pp.tile([8, 4, 512], F32)     # [head, qc, q]
                nc.vector.memset(DSTK[:], 1.0)
                RSTKF = pp.tile([8, 4, 512], F32)    # fp32 reciprocals
                RSTK = pp.tile([8, 4, 512], BF16)    # bf16 for the matmul
                # SEL[:, jb, m]: one-hot picking recip row 2jb (m<64) or
                # 2jb+1 (m>=64) -> BC = SEL.T @ R broadcasts both heads
                SEL = pp.tile([8, 4, 128], BF16)
                nc.sync.dma_start(
                    out=SEL[:].rearrange("p a b -> p (a b)"), in_=sel[:])
                # out-proj weights (loaded after startup-critical DMAs)
                WO = pp.tile([128, 4, D], BF16)

                # --- fused pipeline, tn-major: proj(tn) -> norm+
                # outproj(qc=tn-1) -> attention(qc=tn).  PSUM budget
                # (8 banks): sc 2x2 + pv 2x1 + shared "u" ring 2x1.
                with tc.tile_pool(name="px", bufs=1) as px, \
                     tc.tile_pool(name="pt", bufs=10) as ptp, \
                     tc.tile_pool(name="po", bufs=3) as po, \
                     tc.tile_pool(name="pm", bufs=2, space="PSUM") as pm, \
                     tc.tile_pool(name="pf", bufs=2, space="PSUM") as pf, \
                     tc.tile_pool(name="ppv", bufs=2, space="PSUM") as ppv:
                    WQ = px.tile([128, 8, EH], BF16)

                    def load_wq():
                        # 2-kc-wide DMAs: half the queue-side overhead
                        for i, kc in enumerate(range(0, 8, 2)):
                            eng = (nc.scalar, nc.sync,
                                   nc.gpsimd)[(i + 1) % 3]
                            eng.dma_start(
                                out=WQ[:, kc : kc + 2, :],
                                in_=wq[kc * 128 : (kc + 2) * 128,
                                       :].rearrange(
                                    "(j p) e -> p j e", j=2))

                    def warmup():
                        # ~4.5us of dummy matmuls: warms the PE HAM
                        # clock gate (1.2 -> 2.4 GHz) while the x/W_Q
                        # DMAs stream in
                        wt = pf.tile([128, 128], F32, tag="f")
                        for i in range(36):
                            nc.tensor.matmul(
                                wt[:], IDN[:], IDN[:],
                                start=True, stop=True)

                    def proj_eb(tn, XT, eb):
                        ps = pf.tile([128, 512], F32, tag="f")
                        for kc in range(8):
                            nc.tensor.matmul(
                                ps[:],
                                WQ[:, kc, eb * 128 : (eb + 1) * 128],
                                XT[:, kc, :],
                                start=(kc == 0), stop=(kc == 7))
                        nc.vector.tensor_copy(
                            QKVT[:, eb, tn * 512 : (tn + 1) * 512],
                            ps[:])
                        # V = PE-transposed qkvT blocks for this e-block's
                        # 4 t-chunks (batched into one bank slot, one DVE
                        # eviction into the ones-augmented VA layout)
                        tp4 = pf.tile([128, 4, 128], BF16, tag="f")
                        for ti in range(4):
                            tb = 4 * tn + ti
                            nc.tensor.transpose(
                                tp4[:, ti, :],
                                QKVT[:, eb, tb * 128 : (tb + 1) * 128],
                                IDN[:])
                        nc.vector.tensor_copy(
                            VA[:, 4 * tn : 4 * tn + 4,
                               2 * eb : 2 * eb + 2, 0:DH],
                            tp4[:].rearrange("p t (h d) -> p t h d", h=2))

                    def proj(tn, XT):
                        for eb in range(4):
                            proj_eb(tn, XT, eb)

                    def attention(qc, jbs=range(4)):
                        qs = qc * 512
                        nkb = 4 * qc + 4
                        for jb in jbs:           # head pair block
                            qA = QKVT[0:64, jb, :]    # head 2jb  [64, S]
                            qB = QKVT[64:128, jb, :]  # head 2jb+1
                            CA = ppv.tile([65, 512], F32, tag="pv")
                            CB = ppv.tile([65, 512], F32, tag="pv")
                            pts = []

                            def pv_emit(kb):
                                pt, po_, n = pts[kb]
                                for hh, C in ((0, CA), (1, CB)):
                                    nc.tensor.matmul(
                                        C[:, po_ : po_ + n],
                                        VA[:, kb, 2 * jb + hh, :],
                                        pt[:, hh, :],
                                        start=(kb == 0),
                                        stop=(kb == nkb - 1),
                                        skip_group_check=True)

                            for kb in range(nkb):
                                k0 = kb * 128
                                q0 = max(k0, qs)
                                n = qs + 512 - q0
                                po_ = q0 - qs
                                sc = pm.tile([128, 1024], F32, tag="m")
                                nc.tensor.matmul(
                                    sc[:, 0:n],
                                    qA[:, k0 : k0 + 128],
                                    qA[:, q0 : q0 + n],
                                    start=True, stop=True)
                                nc.tensor.matmul(
                                    sc[:, 512 : 512 + n],
                                    qB[:, k0 : k0 + 128],
                                    qB[:, q0 : q0 + n],
                                    start=True, stop=True)
                                pt = ptp.tile([128, 2, n], BF16, tag="pt")
                                scv = sc[:].rearrange(
                                    "p (two n) -> p two n", two=2)[
                                    :, :, 0:n]
                                # offload some exp blocks to DVE via a
                                # Schraudolph bit-trick so ACT and DVE
                                # run concurrently: every 5th full block
                                # plus half the diagonal blocks
                                use_dve = qc >= 1 and kb >= 4 * qc
                                if use_dve:
                                    nc.vector.tensor_scalar(
                                        out=pt[:].bitcast(I16),
                                        in0=scv,
                                        scalar1=SCHRA_A, scalar2=SCHRA_B,
                                        op0=MULT, op1=ADD)
                                else:
                                    nc.scalar.activation(
                                        pt[:], scv, EXP, scale=0.125)
                                if k0 >= qs:   # diagonal 128x128 block
                                    nc.vector.tensor_mul(
                                        pt[:, :, 0:128], pt[:, :, 0:128],
                                        MASK2[:])
                                pts.append((pt, po_, n))
                                if kb >= 1:
                                    pv_emit(kb - 1)
                            pv_emit(nkb - 1)
                            for hh, C in ((0, CA), (1, CB)):
                                h = 2 * jb + hh
                                slot = hh * 2 + (jb & 1)
                                nc.vector.tensor_copy(
                                    DEN64[64:65, slot, :], C[64:65, :])
                                nc.sync.dma_start(
                                    out=DSTK[h : h + 1, qc, :],
                                    in_=DEN64[64:65, slot, :])
                            for hh, C in ((0, CA), (1, CB)):
                                nc.vector.tensor_copy(
                                    CTXT[hh * 64 : hh * 64 + 64, jb,
                                         qs : qs + 512],
                                    C[0:64, :])

                    def norm_jb(qc, jb):
                        BC = pf.tile([128, 512], F32, tag="f")
                        nc.tensor.matmul(
                            BC[:],
                            SEL[:, jb, :],
                            RSTK[:, qc, :],
                            start=True, stop=True)
                        dst = CTXT[:, jb, qc * 512 : qc * 512 + 512]
                        nc.vector.tensor_mul(dst, dst, BC[:])

                    def norm_outproj(qc):
                        if True:
                            with nc.allow_low_precision(
                                    reason="approx recip, bf16 norm ok"):
                                nc.vector.reciprocal_approx_fast(
                                    out=RSTKF[:, qc, :],
                                    in_=DSTK[:, qc, :])
                                nc.vector.tensor_copy(
                                    RSTK[:, qc, :], RSTKF[:, qc, :])
                            for jb in range(4):
                                norm_jb(qc, jb)
                        for tb in range(4 * qc, 4 * qc + 4):
                            for nn in range(2):
                                ps = pf.tile([128, 512], F32, tag="f")
                                for eb in range(4):
                                    nc.tensor.matmul(
                                        ps[:],
                                        CTXT[:, eb,
                                             tb * 128 : (tb + 1) * 128],
                                        WO[:, eb,
                                           nn * 512 : (nn + 1) * 512],
                                        start=(eb == 0), stop=(eb == 3))
                                ob = po.tile([128, 512], BF16, tag="ob")
                                nc.vector.tensor_copy(ob[:], ps[:])
                                # alternate queues so the final chunks
                                # drain in parallel (3-way for the last
                                # chunk: ACT is idle at the tail)
                                if qc == 3:
                                    oeng = (nc.sync, nc.gpsimd,
                                            nc.scalar)[(2 * tb + nn) % 3]
                                else:
                                    oeng = nc.sync if (tb + nn) % 2 == 0 \
                                        else nc.gpsimd
                                oeng.dma_start(
                                    out=outp[tb * 128 : (tb + 1) * 128,
                                             nn * 512 : (nn + 1) * 512],
                                    in_=ob[:])

                    def load_x(tn):
                        XT = px.tile([128, 8, 512], BF16, tag="xt",
                                     bufs=2, name=f"xt{tn}")
                        for i, kc in enumerate(range(0, 8, 2)):
                            if tn == 0:
                                # three queues: minimize time-to-first-MM
                                eng = (nc.gpsimd, nc.scalar,
                                       nc.sync)[i % 3]
                            else:
                                eng = nc.sync if i % 2 == 0 \
                                    else nc.gpsimd
                            eng.dma_start(
                                out=XT[:, kc : kc + 2, :],
                                in_=xT[kc * 128 : (kc + 2) * 128,
                                       tn * 512 : (tn + 1) * 512
                                       ].rearrange(
                                    "(j p) t -> p j t", j=2))
                        return XT

                    # issue order = scheduler priority: attention(tn)
                    # first (feeds ACT), then proj(tn+1) and
                    # norm+outproj(tn-1) as PE filler for stalls.
                    XTn = load_x(0)
                    load_wq()
                    XT1 = load_x(1)   # prefetch: slot B is free
                    warmup()
                    # startup: interleave proj(0) with attention(0) per
                    # head-pair (jb needs only e-block jb) so the exp
                    # engines start ~15us earlier
                    for eb in range(4):
                        proj_eb(0, XTn, eb)
                        attention(0, jbs=[eb])
                    for tn in range(4):
                        if tn >= 1:
                            attention(tn)
                        if tn == 0:
                            XTn = XT1
                            proj(1, XTn)
                            for eb in range(4):
                                nc.gpsimd.dma_start(
                                    out=WO[:, eb, :],
                                    in_=wo[eb * 128 : (eb + 1) * 128, :])
                        elif tn < 3:
                            XTn = load_x(tn + 1)
                            proj(tn + 1, XTn)
                        if tn >= 1:
                            norm_outproj(tn - 1)
                    norm_outproj(3)
    nc.compile()
    return nc, {"xT": xT.name, "wq": wq.name, "wo": wo.name,
                "sel": sel.name, "outp": outp.name}


def _get():
    if "nc" not in _CACHE:
        _CACHE["nc"], _CACHE["names"] = _build()
    return _CACHE["nc"], _CACHE["names"]


def _run(x, W_Q, W_out, trace=False):
    import ml_dtypes
    from concourse.bass_utils import run_bass_kernel_spmd

    BF = ml_dtypes.bfloat16
    nc, nm = _get()
    sel = np.zeros((8, 4, 128), np.float32)
    for jb in range(4):
        sel[2 * jb, jb, 0:64] = 1.0
        sel[2 * jb + 1, jb, 64:128] = 1.0
    sel = np.ascontiguousarray(sel.reshape(8, 512)).astype(BF)
    in_maps = []
    for c in range(N_CORES):
        b, hg = c // 2, c % 2
        in_maps.append({
            nm["xT"]: np.ascontiguousarray(x[b].T.astype(BF)),
            nm["wq"]: np.ascontiguousarray(
                W_Q[:, hg * EH : (hg + 1) * EH].astype(BF)),
            nm["wo"]: np.ascontiguousarray(
                W_out[hg * EH : (hg + 1) * EH, :].astype(BF)),
            nm["sel"]: sel,
        })
    return run_bass_kernel_spmd(
        nc, in_maps, list(range(N_CORES)), trace=trace), nm


def kernel(x, W_Q, W_out, b_out):
    res, nm = _run(np.asarray(x), np.asarray(W_Q), np.asarray(W_out))
    bo = np.asarray(b_out, dtype=np.float32)
    out = np.empty((B, S, D), np.float32)
    for b in range(B):
        out[b] = (res.results[2 * b][nm["outp"]].astype(np.float32)
                  + res.results[2 * b + 1][nm["outp"]].astype(np.float32)
                  + bo)
    return out
